# revision 15
# baseline (speedup 1.0000x reference)
"""GCNII (64-layer) + MLP head on 8 Trainium2 NeuronCores.

Strategy (node-sharded graph parallel, bf16 data path):
  - Nodes sharded contiguously across 8 cores (12500 each). Each core owns the
    segment-sum for its destination nodes.
  - The node-feature table (x scaled by dinv[src]) is bf16 and replicated in
    each core's DRAM as pair-tokens ([N/2, 128] bf16, 256B rows), refreshed per
    layer with an AllGather of the per-core shards.
  - Per layer, each core gathers its in-edges' pair-tokens with dma_gather
    (idx = src//2, int16, 2 source ranges of 50k nodes) and aggregates per
    512-destination window via indicator matmuls into PSUM:
    psum[64, 64-wide slice] += g[128e, 64f(parity half)].T @ ind[128e, 64].
    Chunks are parity-pure (the 64-col half of the gathered token selects the
    row), and the 0/1 indicators are host-prebuilt fp8 tiles RESIDENT in SBUF
    for all 64 layers (no per-layer DVE is_equal).
  - The alpha*x0 initial-residual enters PSUM via an identity matmul of
    x0/(0.9*dinv) so the per-window epilogue is a single DVE multiply by
    0.9*dinv[dst], then x = relu(h @ W'l) with W'l = (1-b)I + b*Wl folded on
    the host (bf16), transposed back to node-major, and AllGathered.
  - PSUM column offsets are per-core data (loaded into PE registers from SBUF)
    so a single SPMD program serves all 8 cores; chunk-count templates are
    cross-core maxima.
"""
import os
import numpy as np
import ml_dtypes

# problem dims
N, F, H, L, R, C, E = 100000, 500, 64, 64, 512, 40, 1000000
ALPHA, THETA = 0.1, 0.5
M1, M2 = (R - H) // 3 + H, 2 * ((R - H) // 3) + H  # 213, 362
NCORES = 8
NP = N // NCORES          # 12500
WIN = 512                 # dst window width (psum bank free dim)
NWIN = (NP + WIN - 1) // WIN   # 25 (last window 212 wide)
WBLOCK = 2                # windows per gather-call block
NBLK = (NWIN + WBLOCK - 1) // WBLOCK  # 13
SRCRANGE = 50000          # nodes per src range (tokens <= 32768 for int16)
NRANGE = (N + SRCRANGE - 1) // SRCRANGE  # 2
TOKR = SRCRANGE // 2      # 25000 tokens per range
INDW = 64                 # indicator width (chunk dst span limit)
NLAYERS = int(os.environ.get("GCN_NLAYERS", str(L)))

F32 = np.float32
BF16 = ml_dtypes.bfloat16
FP8 = ml_dtypes.float8_e4m3


def _winw(w):
    return min(WIN, NP - w * WIN)


def preprocess(edge_index):
    """Host-side graph preprocessing. Returns (templates, per-core host data).

    Per core, edges (incl. self-loops) are bucketed by cell = (dst window,
    src range, src parity), sorted by dst, and greedily chunked into <=128-lane
    chunks whose dst span is < INDW. Chunk counts are padded to the cross-core
    max per cell so one SPMD program serves all cores.
    """
    # self-loops are NOT gathered: their dinv*x contribution is injected
    # on-chip via an identity matmul of x*0.9*dinv/0.9 per window.
    src = edge_index[0].astype(np.int64)
    dst = edge_index[1].astype(np.int64)
    deg = (np.bincount(dst, minlength=N) + 1).astype(F32)  # +1 = self loop
    dinv = (1.0 / np.sqrt(deg)).astype(F32)

    NCELL = NWIN * NRANGE * 2
    cores = []
    counts = np.zeros((NCORES, NCELL), np.int64)  # chunks per cell
    for i in range(NCORES):
        m = (dst // NP) == i
        s = src[m]
        d = dst[m] - i * NP
        w = d // WIN
        r = s // SRCRANGE
        p = s & 1
        order = np.lexsort((d, p, r, w))
        s, d, w, r, p = s[order], d[order], w[order], r[order], p[order]
        cell = (w * NRANGE + r) * 2 + p
        ccnt = np.bincount(cell, minlength=NCELL)
        gstart = np.zeros(NCELL + 1, np.int64)
        np.cumsum(ccnt, out=gstart[1:])
        # greedy span-limited chunking per cell
        chunk_lo = []   # per chunk: start edge idx (in sorted core arrays)
        chunk_hi = []
        chunk_cell = []
        for c in range(NCELL):
            lo, hi = gstart[c], gstart[c + 1]
            dc = d[lo:hi]
            k = lo
            while k < hi:
                j = min(k + 128, hi)
                # first index with d >= d[k] + INDW
                ksp = lo + np.searchsorted(dc, d[k] + INDW, side="left")
                j = min(j, max(ksp, k + 1))
                chunk_lo.append(k)
                chunk_hi.append(j)
                chunk_cell.append(c)
                k = j
        chunk_cell = np.asarray(chunk_cell, np.int64)
        counts[i] = np.bincount(chunk_cell, minlength=NCELL)
        cores.append((s, d, np.asarray(chunk_lo), np.asarray(chunk_hi), chunk_cell))

    T = counts.max(axis=0)  # [NCELL] chunk template
    nch = int(T.sum())
    ntot = nch * 128

    # stream order: block -> range -> (window in block -> parity)
    chunk_base = np.zeros(NCELL, np.int64)
    call_chunks = np.zeros((NBLK, NRANGE), np.int64)
    call_c0 = np.zeros((NBLK, NRANGE), np.int64)
    cb = 0
    for b in range(NBLK):
        for r in range(NRANGE):
            call_c0[b][r] = cb
            for w in range(b * WBLOCK, min((b + 1) * WBLOCK, NWIN)):
                for p in range(2):
                    c = (w * NRANGE + r) * 2 + p
                    chunk_base[c] = cb
                    cb += T[c]
            call_chunks[b][r] = cb - call_c0[b][r]
    assert cb == nch

    per_core = []
    for i in range(NCORES):
        s, d, clo, chi, ccell = cores[i]
        idxs = np.zeros(ntot, np.int16)
        ind = np.zeros((ntot, INDW), F32)
        offs = np.zeros(nch, np.int32)
        # per-cell local chunk lists
        cstart = np.zeros(NCELL + 1, np.int64)
        np.cumsum(np.bincount(ccell, minlength=NCELL), out=cstart[1:])
        for c in range(NCELL):
            w = c // (NRANGE * 2)
            r = (c // 2) % NRANGE
            cb0 = chunk_base[c]
            for jl in range(cstart[c], cstart[c + 1]):
                slot = cb0 + (jl - cstart[c])
                lo, hi = clo[jl], chi[jl]
                n = hi - lo
                dloc = d[lo:hi] - w * WIN
                off = int(min(dloc[0], WIN - INDW))
                seg = dloc - off
                assert seg.min() >= 0 and seg.max() < INDW
                offs[slot] = off
                e0 = slot * 128
                idxs[e0:e0 + n] = (s[lo:hi] // 2 - r * TOKR).astype(np.int16)
                ind[np.arange(e0, e0 + n), seg] = 1.0
        eidx = np.tile(idxs.reshape(-1, 16).T, (8, 1))  # [128, ntot/16]
        eind = np.ascontiguousarray(
            ind.reshape(nch, 128, INDW).transpose(1, 0, 2).reshape(128, nch * INDW)
        ).astype(FP8)
        per_core.append(dict(eidx=eidx, eind=eind, eoff=offs.reshape(1, nch),
                             dinv=dinv[i * NP:(i + 1) * NP]))
    meta = dict(T=T, nch=nch, ntot=ntot, chunk_base=chunk_base,
                call_chunks=call_chunks, call_c0=call_c0)
    return meta, per_core


def fold_weights(conv_w):
    betas = np.log(THETA / np.arange(1, L + 1, dtype=F32) + 1.0)
    wp = np.zeros((H, L * H), F32)
    eye = np.eye(H, dtype=F32)
    for l in range(L):
        wp[:, l * H:(l + 1) * H] = (1.0 - betas[l]) * eye + betas[l] * conv_w[l]
    return wp.astype(BF16)


def build_inputs(meta, per_core, inputs):
    """Per-core in_maps for the device program."""
    x_param = np.asarray(inputs["x_param"], F32)
    lin0_w = np.asarray(inputs["lin0_w"], F32)
    lin0_b = np.asarray(inputs["lin0_b"], F32)
    conv_w = np.asarray(inputs["conv_w"], F32)
    wp = fold_weights(conv_w)
    mlp_w0 = np.asarray(inputs["mlp_w0"], F32)
    mlp_w1 = np.asarray(inputs["mlp_w1"], F32)
    mlp_w2 = np.asarray(inputs["mlp_w2"], F32)
    out_w = np.asarray(inputs["out_w"], F32)

    iden2 = np.tile(np.eye(H, dtype=F32), (2, 1)).astype(BF16)   # [128, 64]
    lwt = np.zeros((125, 4, H), F32)
    for k in range(4):
        lwt[:, k, :] = lin0_w[k * 125:(k + 1) * 125, :]
    lb = np.zeros((H, 1), F32)
    lb[:, 0] = lin0_b
    w0d = np.tile(mlp_w0, (2, 1)).astype(BF16)                   # [128, 213]
    mw1a = mlp_w1[:128, :].astype(BF16)
    mw1b = np.zeros((128, M2), F32)
    mw1b[:M1 - 128, :] = mlp_w1[128:, :]
    mw1b = mw1b.astype(BF16)
    mw2 = np.zeros((3, 128, R), F32)
    mw2[0] = mlp_w2[:128]
    mw2[1] = mlp_w2[128:256]
    mw2[2, :M2 - 256] = mlp_w2[256:]
    mw2 = mw2.astype(BF16)
    owt = np.zeros((128, 4, C), F32)
    for mchunk in range(4):
        owt[:, mchunk, :] = out_w[mchunk * 128:(mchunk + 1) * 128, :]
    owt = owt.astype(BF16)
    mb0 = np.zeros((128, 2), F32)
    mb0[:, 0] = np.asarray(inputs["mlp_b0"], F32)[:128]
    mb0[:M1 - 128, 1] = np.asarray(inputs["mlp_b0"], F32)[128:]
    mb1 = np.zeros((128, 3), F32)
    mb1[:, 0] = np.asarray(inputs["mlp_b1"], F32)[:128]
    mb1[:, 1] = np.asarray(inputs["mlp_b1"], F32)[128:256]
    mb1[:M2 - 256, 2] = np.asarray(inputs["mlp_b1"], F32)[256:]
    mb2 = np.zeros((128, 4), F32)
    for mchunk in range(4):
        mb2[:, mchunk] = np.asarray(inputs["mlp_b2"], F32)[mchunk * 128:(mchunk + 1) * 128]
    ob = np.tile(np.asarray(inputs["out_b"], F32), (128, 1))     # [128, 40]

    in_maps = []
    for i in range(NCORES):
        pc = per_core[i]
        dv = pc["dinv"]
        # node-major per-128-group columns: [:, j] = dinv of group j (wd build
        # and tail scale); [:, 98+j] = (ALPHA/0.9)*sqrt(deg) (x0 psum scale)
        dn = np.ones((128, 2 * 98), F32)
        full = (NP // 128) * 128
        dn[:, :NP // 128] = dv[:full].reshape(-1, 128).T
        dn[:NP - full, NP // 128] = dv[full:]
        dn[NP - full:, NP // 128] = 1.0
        inv = (ALPHA / 0.9) / dv  # = (alpha/0.9)*sqrt(deg)
        dn[:, 98:98 + NP // 128] = inv[:full].reshape(-1, 128).T
        dn[:NP - full, 98 + NP // 128] = inv[full:]
        dn[NP - full:, 98 + NP // 128] = 0.0
        in_maps.append({
            "eidx": pc["eidx"], "eind": pc["eind"], "eoff": pc["eoff"],
            "xp": x_param[i * NP:(i + 1) * NP],
            "dinv_nm": dn, "iden2": iden2,
            "wp": wp, "lwt": lwt.reshape(125, 4 * H), "lb": lb,
            "w0d": w0d, "mw1a": mw1a, "mw1b": mw1b,
            "mw2a": mw2[0], "mw2b": mw2[1], "mw2c": mw2[2],
            "owt": owt.reshape(128, 4 * C), "mb0": mb0, "mb1": mb1,
            "mb2": mb2, "ob": ob,
        })
    return in_maps


def build_program(meta, nlayers=NLAYERS):
    import concourse.bass as bass
    import concourse.bacc as bacc
    import concourse.mybir as mybir
    import concourse.tile as tile
    from concourse.bass import ds
    from concourse.masks import make_identity

    T = meta["T"]
    nch = meta["nch"]
    ntot = meta["ntot"]
    chunk_base = meta["chunk_base"]
    call_chunks = meta["call_chunks"]
    call_c0 = meta["call_c0"]
    PE = mybir.EngineType.PE
    f32 = mybir.dt.float32
    bf16 = mybir.dt.bfloat16
    fp8 = mybir.dt.float8e4
    AF = mybir.ActivationFunctionType
    ALU = mybir.AluOpType

    nc = bacc.Bacc("TRN2", target_bir_lowering=False, debug=False,
                   num_devices=NCORES, num_swdge_queues=4)
    # ---- I/O ----
    eidx_in = nc.declare_dram_parameter("eidx", [128, ntot // 16], mybir.dt.int16, isOutput=False)
    eind_in = nc.declare_dram_parameter("eind", [128, nch * INDW], fp8, isOutput=False)
    eoff_in = nc.declare_dram_parameter("eoff", [1, nch], mybir.dt.int32, isOutput=False)
    xp_in = nc.declare_dram_parameter("xp", [NP, F], f32, isOutput=False)
    dinv_in = nc.declare_dram_parameter("dinv_nm", [128, 2 * 98], f32, isOutput=False)
    iden2_in = nc.declare_dram_parameter("iden2", [128, H], bf16, isOutput=False)
    wp_in = nc.declare_dram_parameter("wp", [H, L * H], bf16, isOutput=False)
    lwt_in = nc.declare_dram_parameter("lwt", [125, 4 * H], f32, isOutput=False)
    lb_in = nc.declare_dram_parameter("lb", [H, 1], f32, isOutput=False)
    w0d_in = nc.declare_dram_parameter("w0d", [128, M1], bf16, isOutput=False)
    mw1a_in = nc.declare_dram_parameter("mw1a", [128, M2], bf16, isOutput=False)
    mw1b_in = nc.declare_dram_parameter("mw1b", [128, M2], bf16, isOutput=False)
    mw2a_in = nc.declare_dram_parameter("mw2a", [128, R], bf16, isOutput=False)
    mw2b_in = nc.declare_dram_parameter("mw2b", [128, R], bf16, isOutput=False)
    mw2c_in = nc.declare_dram_parameter("mw2c", [128, R], bf16, isOutput=False)
    owt_in = nc.declare_dram_parameter("owt", [128, 4 * C], bf16, isOutput=False)
    mb0_in = nc.declare_dram_parameter("mb0", [128, 2], f32, isOutput=False)
    mb1_in = nc.declare_dram_parameter("mb1", [128, 3], f32, isOutput=False)
    mb2_in = nc.declare_dram_parameter("mb2", [128, 4], f32, isOutput=False)
    ob_in = nc.declare_dram_parameter("ob", [128, C], f32, isOutput=False)
    out_d = nc.declare_dram_parameter("out", [NP, C], f32, isOutput=True)

    # bf16 node tables: slab = this core's shard, xfull = all-gathered table
    xfull = [nc.dram_tensor(f"xfull{p}", [N // 2, 2 * H], bf16, addr_space="Shared")
             for p in range(2)]
    slab = [nc.dram_tensor(f"slab{p}", [NP, H], bf16) for p in range(2)]

    NJ = 98  # node-major 128-subchunks (last = 84 rows)

    def xfin_loc(w):
        return 64 * (w % 2), (w // 2) * WIN

    def cell_id(w, r, p):
        return (w * NRANGE + r) * 2 + p

    with tile.TileContext(nc) as tc:
        with (
            tc.tile_pool(name="cst", bufs=1) as cst,
        ):
            # ---- resident tiles ----
            eidx = cst.tile([128, ntot // 16], mybir.dt.int16)
            eind = cst.tile([128, nch * INDW], fp8)
            eoff = cst.tile([1, nch], mybir.dt.int32)
            dinv_nm = cst.tile([128, 2 * 98], f32)
            iden2 = cst.tile([128, H], bf16)
            iden = cst.tile([128, 128], f32)
            wp = cst.tile([H, L * H], bf16)
            # wdx rows 0:64 = x0/(0.9*dinv) (psum seed), 64:128 = 0.9*dinv
            wdx = cst.tile([128, NP], bf16)
            x_fin = cst.tile([128, ((NWIN + 1) // 2) * WIN], bf16)
            lwt = cst.tile([125, 4 * H], f32)
            lb = cst.tile([H, 1], f32)
            ones09 = cst.tile([128, H], f32)

            nc.sync.dma_start(eidx[:], eidx_in[:])
            nc.sync.dma_start(eind[:], eind_in[:])
            nc.sync.dma_start(eoff[:], eoff_in[:])
            nc.sync.dma_start(dinv_nm[:], dinv_in[:])
            nc.sync.dma_start(iden2[:], iden2_in[:])
            nc.sync.dma_start(wp[:], wp_in[:])
            nc.sync.dma_start(lwt[:], lwt_in[:])
            nc.sync.dma_start(lb[:], lb_in[:])
            make_identity(nc, iden[:])
            nc.gpsimd.memset(ones09[:], 0.9)

            def tail_window(w, stpool, out_par, psC):
                """x_fin window -> transpose -> dinv scale -> slab[out_par]."""
                xb, xc = xfin_loc(w)
                winw = _winw(w)
                stage = stpool.tile([128, 4 * H], bf16, tag="stage")
                ng = (winw + 127) // 128
                for g in range(ng):
                    gn = min(128, winw - g * 128)
                    pst = psC.tile([128, H], bf16, space="PSUM", tag="psC")
                    nc.tensor.transpose(
                        pst[:gn, :],
                        x_fin[xb:xb + H, xc + g * 128: xc + g * 128 + gn],
                        iden2[xb:xb + H, :],
                    )
                    nc.scalar.activation(
                        stage[:gn, g * H:(g + 1) * H], pst[:gn, :], AF.Copy,
                        scale=dinv_nm[:gn, (4 * w + g):(4 * w + g) + 1],
                    )
                base = w * WIN
                if winw == WIN:
                    nc.sync.dma_start(
                        slab[out_par][base:base + WIN, :].rearrange(
                            "(g p) f -> p g f", p=128),
                        stage[:].rearrange("p (g f) -> p g f", f=H),
                    )
                else:
                    for g in range(ng):
                        gn = min(128, winw - g * 128)
                        nc.sync.dma_start(
                            slab[out_par][base + g * 128:base + g * 128 + gn, :],
                            stage[:gn, g * H:(g + 1) * H],
                        )

            def allgather(out_par):
                nc.gpsimd.collective_compute(
                    "AllGather", ALU.bypass,
                    replica_groups=[list(range(NCORES))],
                    ins=[slab[out_par][:]],
                    outs=[xfull[out_par][:]],
                )

            # ================= init: wdx, x0, first slab =================
            with (
                tc.tile_pool(name="initp", bufs=2) as initp,
                tc.tile_pool(name="initw", bufs=3) as initw,
                tc.tile_pool(name="ipsB", bufs=2, space="PSUM") as psB,
                tc.tile_pool(name="ipsC", bufs=2, space="PSUM") as psC,
            ):
                # wd = 0.9 * dinv broadcast over 64 partitions -> wdx[64:]
                for j in range(NJ):
                    gn = min(128, NP - j * 128)
                    diag = initw.tile([128, 128], f32, tag="diag")
                    nc.vector.tensor_scalar_mul(
                        diag[:], iden[:], dinv_nm[:, j:j + 1])
                    pw = psB.tile([H, 128], f32, space="PSUM", tag="psB")
                    nc.tensor.matmul(pw[:, :gn], lhsT=ones09[:],
                                     rhs=diag[:, :gn], start=True, stop=True)
                    nc.scalar.activation(
                        wdx[H:, j * 128:j * 128 + gn], pw[:, :gn], AF.Copy)
                # x0 = relu(xp @ lin0_w + b)
                for c in range(NWIN):
                    winw = _winw(c)
                    ng = (winw + 127) // 128
                    xpt = initp.tile([128, 4 * F], f32, tag="xpt")
                    base = c * WIN
                    if winw == WIN:
                        nc.sync.dma_start(
                            xpt[:].rearrange("p (g f) -> p g f", f=F),
                            xp_in[base:base + WIN, :].rearrange(
                                "(g p) f -> p g f", p=128),
                        )
                    else:
                        for g in range(ng):
                            gn = min(128, winw - g * 128)
                            nc.sync.dma_start(
                                xpt[:gn, g * F:(g + 1) * F],
                                xp_in[base + g * 128:base + g * 128 + gn, :],
                            )
                    px0 = psB.tile([H, WIN], f32, space="PSUM", tag="psB")
                    for k in range(4):
                        xt = initw.tile([125, WIN], f32, tag="xt")
                        for g in range(ng):
                            gn = min(128, winw - g * 128)
                            ptr = psC.tile([128, 128], f32, space="PSUM",
                                           tag="psC")
                            nc.tensor.transpose(
                                ptr[:125, :gn],
                                xpt[:gn, g * F + 125 * k: g * F + 125 * (k + 1)],
                                iden[:gn, :gn],
                            )
                            nc.scalar.activation(
                                xt[:, g * 128:g * 128 + gn], ptr[:125, :gn],
                                AF.Copy)
                        nc.tensor.matmul(
                            px0[:, :winw], lhsT=lwt[:, k * H:(k + 1) * H],
                            rhs=xt[:, :winw], start=(k == 0), stop=(k == 3))
                    xb, xc = xfin_loc(c)
                    nc.scalar.activation(
                        x_fin[xb:xb + H, xc:xc + winw], px0[:, :winw],
                        AF.Relu, bias=lb[:, 0:1])
                    # x0scale = (alpha/0.9)*sqrt(deg) broadcast over H rows
                    # (host bakes (alpha/0.9)/dinv; ones09 contributes 0.9,
                    # divided back out via the activation scale)
                    x0w = initw.tile([128, WIN], bf16, tag="x0w")
                    for g in range(ng):
                        gn = min(128, winw - g * 128)
                        diag2 = initw.tile([128, 128], f32, tag="diag")
                        nc.vector.tensor_scalar_mul(
                            diag2[:], iden[:],
                            dinv_nm[:, 98 + 4 * c + g:98 + 4 * c + g + 1])
                        pw2 = psB.tile([H, 128], f32, space="PSUM", tag="psB")
                        nc.tensor.matmul(pw2[:, :gn], lhsT=ones09[:],
                                         rhs=diag2[:, :gn], start=True,
                                         stop=True)
                        nc.scalar.activation(
                            x0w[xb:xb + H, g * 128:g * 128 + gn], pw2[:, :gn],
                            AF.Copy, scale=1.0 / 0.9)
                    # wdx[:64, win] = x0 * (alpha/0.9)*sqrt(deg)  (psum seed)
                    nc.vector.tensor_tensor(
                        wdx[:H, base:base + winw],
                        x_fin[xb:xb + H, xc:xc + winw],
                        x0w[xb:xb + H, :winw], op=ALU.mult)
                for w in range(NWIN):
                    tail_window(w, initw, 0, psC)
                allgather(0)

            # ================= layers =================
            with (
                tc.tile_pool(name="gp", bufs=4) as gpool,
                tc.tile_pool(name="hp", bufs=2) as hpool,
                tc.tile_pool(name="stp", bufs=2) as stpool,
                tc.tile_pool(name="psA", bufs=4, space="PSUM") as psA,
                tc.tile_pool(name="psB", bufs=2, space="PSUM") as psB,
                tc.tile_pool(name="psC", bufs=2, space="PSUM") as psC,
            ):
                cbmax = int(call_chunks.max())
                GPBUFS, PSABUFS = 4, 4
                g_hist = []
                g_hist_idx = {}
                psw_hist = []
                callno = 0
                for lay in range(nlayers):
                    par = lay % 2
                    for b in range(NBLK):
                        wlist = list(range(b * WBLOCK,
                                           min((b + 1) * WBLOCK, NWIN)))
                        gt = {}
                        gt_last = {}
                        for r in range(NRANGE):
                            cc = int(call_chunks[b][r])
                            if cc == 0:
                                continue
                            c0 = int(call_c0[b][r])
                            g = gpool.tile([128, cbmax * 128], bf16, tag="gath")
                            toks = min(TOKR, N // 2 - r * TOKR)
                            gref = nc.gpsimd.dma_gather(
                                out_ap=g[:, :cc * 128].rearrange(
                                    "p (c d) -> p c d", d=128),
                                in_ap=xfull[par][r * TOKR:r * TOKR + toks, :],
                                idxs_ap=eidx[:, c0 * 8:(c0 + cc) * 8],
                                num_idxs=cc * 128,
                                num_idxs_reg=cc * 128,
                                elem_size=2 * H,
                                single_packet=False,
                                queue_num=0 if os.environ.get("GCN_1Q")
                                else callno % 4,
                            )
                            callno += 1
                            if len(g_hist) >= GPBUFS and g_hist[-GPBUFS] is not None:
                                tile.add_dep_helper(gref.ins, g_hist[-GPBUFS].ins,
                                                    sync=False)
                            g_hist.append(None)  # placeholder, set after matmuls
                            g_hist_idx[id(g)] = len(g_hist) - 1
                            gt[r] = (g, c0)
                        pswt = {}
                        wleft = {w: sum(int(T[cell_id(w, r, p)])
                                        for r in range(NRANGE) for p in range(2))
                                 for w in wlist}
                        # seed psum with the alpha*x0 residual (identity mm)
                        for w in wlist:
                            winw = _winw(w)
                            psw = psA.tile([H, WIN], f32, space="PSUM",
                                           tag="psw", name=f"psw{w}")
                            pswt[w] = psw
                            mm = nc.tensor.matmul(
                                psw[:, :winw], lhsT=iden2[:H, :],
                                rhs=wdx[:H, w * WIN:w * WIN + winw],
                                start=True, stop=(wleft[w] == 0))
                            if len(psw_hist) >= PSABUFS:
                                tile.add_dep_helper(
                                    mm.ins, psw_hist[-PSABUFS].ins, sync=False)
                        for r in range(NRANGE):
                            if r not in gt:
                                continue
                            g, gchunk0 = gt[r]
                            for w in wlist:
                                psw = pswt[w]
                                for p in range(2):
                                    t = int(T[cell_id(w, r, p)])
                                    if t == 0:
                                        continue
                                    c0 = int(chunk_base[cell_id(w, r, p)])
                                    jj0 = c0 - gchunk0
                                    _, offv = nc.values_load_multi_w_load_instructions(
                                        eoff[0:1, c0:c0 + t], engines=[PE],
                                        min_val=0, max_val=WIN - INDW,
                                        skip_runtime_bounds_check=True,
                                    )
                                    for j in range(t):
                                        wleft[w] -= 1
                                        mm = nc.tensor.matmul(
                                            psw[:, ds(offv[j], INDW)],
                                            lhsT=g[:, (jj0 + j) * 128 + H * p:
                                                   (jj0 + j) * 128 + H * (p + 1)],
                                            rhs=eind[:, (c0 + j) * INDW:
                                                     (c0 + j + 1) * INDW],
                                            start=False,
                                            stop=(wleft[w] == 0),
                                        )
                                        gt_last[r] = mm
                        for w in wlist:
                            winw = _winw(w)
                            psw = pswt[w]
                            # hh = psw * 0.9*dinv ; x = relu(hh @ W'l)
                            hh = hpool.tile([H, WIN], bf16, tag="hh")
                            hm = nc.vector.tensor_tensor(
                                hh[:, :winw], psw[:, :winw],
                                wdx[H:, w * WIN:w * WIN + winw], op=ALU.mult)
                            psw_hist.append(hm)
                            ps2 = psB.tile([H, WIN], f32, space="PSUM",
                                           tag="psB")
                            nc.tensor.matmul(
                                ps2[:, :winw],
                                lhsT=wp[:, lay * H:(lay + 1) * H],
                                rhs=hh[:, :winw], start=True, stop=True)
                            xb, xc = xfin_loc(w)
                            nc.scalar.activation(
                                x_fin[xb:xb + H, xc:xc + winw],
                                ps2[:, :winw], AF.Relu)
                        for r in range(NRANGE):
                            if r in gt:
                                g, _ = gt[r]
                                g_hist[g_hist_idx[id(g)]] = gt_last.get(r)
                        if lay < nlayers - 1:
                            for w in wlist:
                                tail_window(w, stpool, 1 - par, psC)
                    if lay < nlayers - 1:
                        allgather(1 - par)

            # ================= MLP head (bf16) =================
            with (
                tc.tile_pool(name="mh", bufs=2) as mh,
                tc.tile_pool(name="mw", bufs=1) as mw,
                tc.tile_pool(name="mpsA", bufs=4, space="PSUM") as psA,
                tc.tile_pool(name="mpsB", bufs=2, space="PSUM") as psB,
                tc.tile_pool(name="mpsC", bufs=2, space="PSUM") as psC,
            ):
                w0d = mw.tile([128, M1], bf16)
                mw1a = mw.tile([128, M2], bf16)
                mw1b = mw.tile([128, M2], bf16)
                mw2a = mw.tile([128, R], bf16)
                mw2b = mw.tile([128, R], bf16)
                mw2c = mw.tile([128, R], bf16)
                owt = mw.tile([128, 4 * C], bf16)
                mb0 = mw.tile([128, 2], f32)
                mb1 = mw.tile([128, 3], f32)
                mb2 = mw.tile([128, 4], f32)
                ob = mw.tile([128, C], f32)
                nc.sync.dma_start(w0d[:], w0d_in[:])
                nc.sync.dma_start(mw1a[:], mw1a_in[:])
                nc.sync.dma_start(mw1b[:], mw1b_in[:])
                nc.sync.dma_start(mw2a[:], mw2a_in[:])
                nc.sync.dma_start(mw2b[:], mw2b_in[:])
                nc.sync.dma_start(mw2c[:], mw2c_in[:])
                nc.sync.dma_start(owt[:], owt_in[:])
                nc.sync.dma_start(mb0[:], mb0_in[:])
                nc.sync.dma_start(mb1[:], mb1_in[:])
                nc.sync.dma_start(mb2[:], mb2_in[:])
                nc.sync.dma_start(ob[:], ob_in[:])
                for c in range(NWIN):
                    winw = _winw(c)
                    xb, xc = xfin_loc(c)
                    xin = x_fin[xb:xb + H, xc:xc + winw]
                    # h1 = relu(x @ w0 + b0): [213, winw]
                    h1a = mh.tile([128, WIN], bf16, tag="h1a")
                    h1b = mh.tile([M1 - 128, WIN], bf16, tag="h1b")
                    p1 = psB.tile([128, WIN], f32, space="PSUM", tag="psB")
                    nc.tensor.matmul(p1[:, :winw], lhsT=w0d[xb:xb + H, :128],
                                     rhs=xin, start=True, stop=True)
                    nc.scalar.activation(h1a[:, :winw], p1[:, :winw], AF.Relu,
                                         bias=mb0[:, 0:1])
                    p1b = psB.tile([M1 - 128, WIN], f32, space="PSUM",
                                   tag="psB")
                    nc.tensor.matmul(p1b[:, :winw],
                                     lhsT=w0d[xb:xb + H, 128:M1],
                                     rhs=xin, start=True, stop=True)
                    nc.scalar.activation(h1b[:, :winw], p1b[:, :winw], AF.Relu,
                                         bias=mb0[:M1 - 128, 1:2])
                    # h2 = relu(h1 @ w1 + b1): [362, winw]
                    h2 = []
                    for mchunk in range(3):
                        mn = min(128, M2 - mchunk * 128)
                        p2 = psA.tile([128, WIN], f32, space="PSUM", tag="psw")
                        nc.tensor.matmul(
                            p2[:mn, :winw],
                            lhsT=mw1a[:, mchunk * 128:mchunk * 128 + mn],
                            rhs=h1a[:, :winw], start=True, stop=False)
                        nc.tensor.matmul(
                            p2[:mn, :winw],
                            lhsT=mw1b[:M1 - 128, mchunk * 128:mchunk * 128 + mn],
                            rhs=h1b[:M1 - 128, :winw], start=False, stop=True)
                        h2t = mh.tile([128, WIN], bf16, tag=f"h2_{mchunk}")
                        nc.scalar.activation(h2t[:mn, :winw], p2[:mn, :winw],
                                             AF.Relu,
                                             bias=mb1[:mn, mchunk:mchunk + 1])
                        h2.append(h2t)
                    # h3 = h2 @ w2 + b2 (no relu): [512, winw]
                    h3 = []
                    w2t = [mw2a, mw2b, mw2c]
                    kn = [128, 128, M2 - 256]
                    for mchunk in range(4):
                        p3 = psA.tile([128, WIN], f32, space="PSUM", tag="psw")
                        for k in range(3):
                            nc.tensor.matmul(
                                p3[:, :winw],
                                lhsT=w2t[k][:kn[k],
                                            mchunk * 128:(mchunk + 1) * 128],
                                rhs=h2[k][:kn[k], :winw],
                                start=(k == 0), stop=(k == 2))
                        h3t = mh.tile([128, WIN], bf16, tag=f"h3_{mchunk}")
                        nc.scalar.activation(h3t[:, :winw], p3[:, :winw],
                                             AF.Identity,
                                             bias=mb2[:, mchunk:mchunk + 1])
                        h3.append(h3t)
                    # out = h3 @ out_w + out_b: node-major [winw, 40]
                    ost = mh.tile([128, 4 * C], f32, tag="ost")
                    ng = (winw + 127) // 128
                    for g in range(ng):
                        gn = min(128, winw - g * 128)
                        po = psC.tile([128, C], f32, space="PSUM", tag="psC")
                        for k in range(4):
                            nc.tensor.matmul(
                                po[:gn, :],
                                lhsT=h3[k][:, g * 128:g * 128 + gn],
                                rhs=owt[:, k * C:(k + 1) * C],
                                start=(k == 0), stop=(k == 3))
                        nc.vector.tensor_tensor(
                            ost[:gn, g * C:(g + 1) * C], po[:gn, :],
                            ob[:gn, :], op=ALU.add)
                    base = c * WIN
                    if winw == WIN:
                        nc.sync.dma_start(
                            out_d[base:base + WIN, :].rearrange(
                                "(g p) f -> p g f", p=128),
                            ost[:].rearrange("p (g f) -> p g f", f=C),
                        )
                    else:
                        for g in range(ng):
                            gn = min(128, winw - g * 128)
                            nc.sync.dma_start(
                                out_d[base + g * 128:base + g * 128 + gn, :],
                                ost[:gn, g * C:(g + 1) * C],
                            )
    nc.finalize()
    return nc


_CACHE = {}
_LAST_EXEC_NS = None
_LAST_TRACE = None


def _get_program(meta, nlayers):
    key = (meta["nch"], nlayers, meta["T"].tobytes())
    if key not in _CACHE:
        _CACHE[key] = build_program(meta, nlayers)
    return _CACHE[key]


def kernel(**inputs):
    from concourse.bass_utils import run_bass_kernel_spmd
    edge_index = np.asarray(inputs["edge_index"])
    meta, per_core = preprocess(edge_index)
    in_maps = build_inputs(meta, per_core, inputs)
    nc = _get_program(meta, NLAYERS)
    kwargs = {}
    if os.environ.get("GCN_TRACE"):
        kwargs["trace"] = True
        if os.environ.get("GCN_TRACE_DIR"):
            os.makedirs(os.environ["GCN_TRACE_DIR"], exist_ok=True)
            kwargs["tmpdir"] = os.environ["GCN_TRACE_DIR"]
    res = run_bass_kernel_spmd(nc, in_maps, list(range(NCORES)), **kwargs)
    global _LAST_EXEC_NS, _LAST_TRACE
    if res.exec_time_ns:
        _LAST_EXEC_NS = res.exec_time_ns
        _LAST_TRACE = res.instructions_and_trace
    out = np.concatenate([res.results[i]["out"] for i in range(NCORES)], axis=0)
    return out


# revision 25
# speedup vs baseline: 9.9981x; 9.9981x over previous
"""GCNII (64-layer) + MLP head on 8 Trainium2 NeuronCores.

Strategy (node-sharded graph parallel, bf16 data path):
  - Nodes sharded contiguously across 8 cores (12500 each). Each core owns the
    segment-sum for its destination nodes.
  - The node-feature table (x scaled by dinv[src]) is bf16 and replicated in
    each core's DRAM as pair-tokens ([N/2, 128] bf16, 256B rows), refreshed per
    layer with an AllGather of the per-core shards.
  - Per layer, each core gathers its in-edges' pair-tokens with dma_gather
    (idx = src//2, int16, 2 source ranges of 50k nodes) and aggregates per
    512-destination window via indicator matmuls into PSUM:
    psum[64, 64-wide slice] += g[128e, 64f(parity half)].T @ ind[128e, 64].
    Chunks are parity-pure (the 64-col half of the gathered token selects the
    row), and the 0/1 indicators are host-prebuilt fp8 tiles RESIDENT in SBUF
    for all 64 layers (no per-layer DVE is_equal).
  - The alpha*x0 initial-residual enters PSUM via an identity matmul of
    x0/(0.9*dinv) so the per-window epilogue is a single DVE multiply by
    0.9*dinv[dst], then x = relu(h @ W'l) with W'l = (1-b)I + b*Wl folded on
    the host (bf16), transposed back to node-major, and AllGathered.
  - PSUM column offsets are per-core data (loaded into PE registers from SBUF)
    so a single SPMD program serves all 8 cores; chunk-count templates are
    cross-core maxima.
"""
import os
import numpy as np
import ml_dtypes

# problem dims
N, F, H, L, R, C, E = 100000, 500, 64, 64, 512, 40, 1000000
ALPHA, THETA = 0.1, 0.5
M1, M2 = (R - H) // 3 + H, 2 * ((R - H) // 3) + H  # 213, 362
NCORES = 8
NP = N // NCORES          # 12500
WIN = 512                 # dst window width (psum bank free dim)
NWIN = (NP + WIN - 1) // WIN   # 25 (last window 212 wide)
WBLOCK = 2                # windows per gather-call block
NBLK = (NWIN + WBLOCK - 1) // WBLOCK  # 13
SRCRANGE = 50000          # nodes per src range (tokens <= 32768 for int16)
NRANGE = (N + SRCRANGE - 1) // SRCRANGE  # 2
TOKR = SRCRANGE // 2      # 25000 tokens per range
INDW = 64                 # indicator width (chunk dst span limit)
NLAYERS = int(os.environ.get("GCN_NLAYERS", str(L)))

F32 = np.float32
BF16 = ml_dtypes.bfloat16
FP8 = ml_dtypes.float8_e4m3


def _winw(w):
    return min(WIN, NP - w * WIN)


def preprocess(edge_index):
    """Host-side graph preprocessing. Returns (templates, per-core host data).

    Per core, edges (incl. self-loops) are bucketed by cell = (dst window,
    src range, src parity), sorted by dst, and greedily chunked into <=128-lane
    chunks whose dst span is < INDW. Chunk counts are padded to the cross-core
    max per cell so one SPMD program serves all cores.
    """
    # self-loops are NOT gathered: their dinv*x contribution is injected
    # on-chip via an identity matmul of x*0.9*dinv/0.9 per window.
    src = edge_index[0].astype(np.int64)
    dst = edge_index[1].astype(np.int64)
    deg = (np.bincount(dst, minlength=N) + 1).astype(F32)  # +1 = self loop
    dinv = (1.0 / np.sqrt(deg)).astype(F32)

    NCELL = NWIN * NRANGE * 2
    cores = []
    counts = np.zeros((NCORES, NCELL), np.int64)  # chunks per cell
    for i in range(NCORES):
        m = (dst // NP) == i
        s = src[m]
        d = dst[m] - i * NP
        w = d // WIN
        r = s // SRCRANGE
        p = s & 1
        order = np.lexsort((d, p, r, w))
        s, d, w, r, p = s[order], d[order], w[order], r[order], p[order]
        cell = (w * NRANGE + r) * 2 + p
        ccnt = np.bincount(cell, minlength=NCELL)
        gstart = np.zeros(NCELL + 1, np.int64)
        np.cumsum(ccnt, out=gstart[1:])
        # greedy span-limited chunking per cell
        chunk_lo = []   # per chunk: start edge idx (in sorted core arrays)
        chunk_hi = []
        chunk_cell = []
        for c in range(NCELL):
            lo, hi = gstart[c], gstart[c + 1]
            dc = d[lo:hi]
            k = lo
            while k < hi:
                j = min(k + 128, hi)
                # first index with d >= d[k] + INDW
                ksp = lo + np.searchsorted(dc, d[k] + INDW, side="left")
                j = min(j, max(ksp, k + 1))
                chunk_lo.append(k)
                chunk_hi.append(j)
                chunk_cell.append(c)
                k = j
        chunk_cell = np.asarray(chunk_cell, np.int64)
        counts[i] = np.bincount(chunk_cell, minlength=NCELL)
        cores.append((s, d, np.asarray(chunk_lo), np.asarray(chunk_hi), chunk_cell))

    T = counts.max(axis=0)  # [NCELL] chunk template
    nch = int(T.sum())
    ntot = nch * 128

    # stream order: block -> range -> (window in block -> parity)
    chunk_base = np.zeros(NCELL, np.int64)
    call_chunks = np.zeros((NBLK, NRANGE), np.int64)
    call_c0 = np.zeros((NBLK, NRANGE), np.int64)
    cb = 0
    for b in range(NBLK):
        for r in range(NRANGE):
            call_c0[b][r] = cb
            for w in range(b * WBLOCK, min((b + 1) * WBLOCK, NWIN)):
                for p in range(2):
                    c = (w * NRANGE + r) * 2 + p
                    chunk_base[c] = cb
                    cb += T[c]
            call_chunks[b][r] = cb - call_c0[b][r]
    assert cb == nch

    per_core = []
    for i in range(NCORES):
        s, d, clo, chi, ccell = cores[i]
        idxs = np.zeros(ntot, np.int16)
        ind = np.zeros((ntot, INDW), F32)
        offs = np.zeros(nch, np.int32)
        # per-cell local chunk lists
        cstart = np.zeros(NCELL + 1, np.int64)
        np.cumsum(np.bincount(ccell, minlength=NCELL), out=cstart[1:])
        for c in range(NCELL):
            w = c // (NRANGE * 2)
            r = (c // 2) % NRANGE
            cb0 = chunk_base[c]
            for jl in range(cstart[c], cstart[c + 1]):
                slot = cb0 + (jl - cstart[c])
                lo, hi = clo[jl], chi[jl]
                n = hi - lo
                dloc = d[lo:hi] - w * WIN
                off = int(min(dloc[0], WIN - INDW))
                seg = dloc - off
                assert seg.min() >= 0 and seg.max() < INDW
                offs[slot] = off
                e0 = slot * 128
                idxs[e0:e0 + n] = (s[lo:hi] // 2 - r * TOKR).astype(np.int16)
                ind[np.arange(e0, e0 + n), seg] = 1.0
        eidx = np.tile(idxs.reshape(-1, 16).T, (8, 1))  # [128, ntot/16]
        eind = np.ascontiguousarray(
            ind.reshape(nch, 128, INDW).transpose(1, 0, 2).reshape(128, nch * INDW)
        ).astype(FP8)
        per_core.append(dict(eidx=eidx, eind=eind, eoff=offs.reshape(1, nch),
                             dinv=dinv[i * NP:(i + 1) * NP]))
    meta = dict(T=T, nch=nch, ntot=ntot, chunk_base=chunk_base,
                call_chunks=call_chunks, call_c0=call_c0)
    return meta, per_core


def fold_weights(conv_w):
    betas = np.log(THETA / np.arange(1, L + 1, dtype=F32) + 1.0)
    wp = np.zeros((H, L * H), F32)
    eye = np.eye(H, dtype=F32)
    for l in range(L):
        wp[:, l * H:(l + 1) * H] = (1.0 - betas[l]) * eye + betas[l] * conv_w[l]
    return wp.astype(BF16)


def build_inputs(meta, per_core, inputs):
    """Per-core in_maps for the device program."""
    x_param = np.asarray(inputs["x_param"], F32)
    lin0_w = np.asarray(inputs["lin0_w"], F32)
    lin0_b = np.asarray(inputs["lin0_b"], F32)
    conv_w = np.asarray(inputs["conv_w"], F32)
    wp = fold_weights(conv_w)
    mlp_w0 = np.asarray(inputs["mlp_w0"], F32)
    mlp_w1 = np.asarray(inputs["mlp_w1"], F32)
    mlp_w2 = np.asarray(inputs["mlp_w2"], F32)
    out_w = np.asarray(inputs["out_w"], F32)

    iden2 = np.tile(np.eye(H, dtype=F32), (2, 1)).astype(BF16)   # [128, 64]
    idens = np.tile((1.0 / 0.9) * np.eye(H, dtype=F32), (2, 1)).astype(BF16)
    lwt = np.zeros((125, 4, H), F32)
    for k in range(4):
        lwt[:, k, :] = lin0_w[k * 125:(k + 1) * 125, :]
    lb = np.zeros((H, 1), F32)
    lb[:, 0] = lin0_b
    w0d = np.tile(mlp_w0, (2, 1)).astype(BF16)                   # [128, 213]
    mw1a = mlp_w1[:128, :].astype(BF16)
    mw1b = np.zeros((128, M2), F32)
    mw1b[:M1 - 128, :] = mlp_w1[128:, :]
    mw1b = mw1b.astype(BF16)
    mw2 = np.zeros((3, 128, R), F32)
    mw2[0] = mlp_w2[:128]
    mw2[1] = mlp_w2[128:256]
    mw2[2, :M2 - 256] = mlp_w2[256:]
    mw2 = mw2.astype(BF16)
    owt = np.zeros((128, 4, C), F32)
    for mchunk in range(4):
        owt[:, mchunk, :] = out_w[mchunk * 128:(mchunk + 1) * 128, :]
    owt = owt.astype(BF16)
    mb0 = np.zeros((128, 2), F32)
    mb0[:, 0] = np.asarray(inputs["mlp_b0"], F32)[:128]
    mb0[:M1 - 128, 1] = np.asarray(inputs["mlp_b0"], F32)[128:]
    mb1 = np.zeros((128, 3), F32)
    mb1[:, 0] = np.asarray(inputs["mlp_b1"], F32)[:128]
    mb1[:, 1] = np.asarray(inputs["mlp_b1"], F32)[128:256]
    mb1[:M2 - 256, 2] = np.asarray(inputs["mlp_b1"], F32)[256:]
    mb2 = np.zeros((128, 4), F32)
    for mchunk in range(4):
        mb2[:, mchunk] = np.asarray(inputs["mlp_b2"], F32)[mchunk * 128:(mchunk + 1) * 128]
    ob = np.tile(np.asarray(inputs["out_b"], F32), (128, 1))     # [128, 40]

    in_maps = []
    for i in range(NCORES):
        pc = per_core[i]
        dv = pc["dinv"]
        # node-major per-128-group columns: [:, j] = dinv of group j (wd build
        # and tail scale); [:, 98+j] = (ALPHA/0.9)*sqrt(deg) (x0 psum scale)
        dn = np.ones((128, 2 * 98), F32)
        full = (NP // 128) * 128
        dn[:, :NP // 128] = dv[:full].reshape(-1, 128).T
        dn[:NP - full, NP // 128] = dv[full:]
        dn[NP - full:, NP // 128] = 1.0
        inv = (ALPHA / 0.9) / dv  # = (alpha/0.9)*sqrt(deg)
        dn[:, 98:98 + NP // 128] = inv[:full].reshape(-1, 128).T
        dn[:NP - full, 98 + NP // 128] = inv[full:]
        dn[NP - full:, 98 + NP // 128] = 0.0
        in_maps.append({
            "eidx": pc["eidx"], "eind": pc["eind"], "eoff": pc["eoff"],
            "xp": x_param[i * NP:(i + 1) * NP],
            "dinv_nm": dn, "iden2": iden2, "idens": idens,
            "wp": wp, "lwt": lwt.reshape(125, 4 * H), "lb": lb,
            "w0d": w0d, "mw1a": mw1a, "mw1b": mw1b,
            "mw2a": mw2[0], "mw2b": mw2[1], "mw2c": mw2[2],
            "owt": owt.reshape(128, 4 * C), "mb0": mb0, "mb1": mb1,
            "mb2": mb2, "ob": ob,
        })
    return in_maps


def build_program(meta, nlayers=NLAYERS):
    import concourse.bass as bass
    import concourse.bacc as bacc
    import concourse.mybir as mybir
    import concourse.tile as tile
    from concourse.bass import ds
    from concourse.masks import make_identity

    T = meta["T"]
    nch = meta["nch"]
    ntot = meta["ntot"]
    chunk_base = meta["chunk_base"]
    call_chunks = meta["call_chunks"]
    call_c0 = meta["call_c0"]
    PE = mybir.EngineType.PE
    f32 = mybir.dt.float32
    bf16 = mybir.dt.bfloat16
    fp8 = mybir.dt.float8e4
    AF = mybir.ActivationFunctionType
    ALU = mybir.AluOpType

    nc = bacc.Bacc("TRN2", target_bir_lowering=False, debug=False,
                   num_devices=NCORES, num_swdge_queues=4)
    # ---- I/O ----
    eidx_in = nc.declare_dram_parameter("eidx", [128, ntot // 16], mybir.dt.int16, isOutput=False)
    eind_in = nc.declare_dram_parameter("eind", [128, nch * INDW], fp8, isOutput=False)
    eoff_in = nc.declare_dram_parameter("eoff", [1, nch], mybir.dt.int32, isOutput=False)
    xp_in = nc.declare_dram_parameter("xp", [NP, F], f32, isOutput=False)
    dinv_in = nc.declare_dram_parameter("dinv_nm", [128, 2 * 98], f32, isOutput=False)
    iden2_in = nc.declare_dram_parameter("iden2", [128, H], bf16, isOutput=False)
    idens_in = nc.declare_dram_parameter("idens", [128, H], bf16, isOutput=False)
    wp_in = nc.declare_dram_parameter("wp", [H, L * H], bf16, isOutput=False)
    lwt_in = nc.declare_dram_parameter("lwt", [125, 4 * H], f32, isOutput=False)
    lb_in = nc.declare_dram_parameter("lb", [H, 1], f32, isOutput=False)
    w0d_in = nc.declare_dram_parameter("w0d", [128, M1], bf16, isOutput=False)
    mw1a_in = nc.declare_dram_parameter("mw1a", [128, M2], bf16, isOutput=False)
    mw1b_in = nc.declare_dram_parameter("mw1b", [128, M2], bf16, isOutput=False)
    mw2a_in = nc.declare_dram_parameter("mw2a", [128, R], bf16, isOutput=False)
    mw2b_in = nc.declare_dram_parameter("mw2b", [128, R], bf16, isOutput=False)
    mw2c_in = nc.declare_dram_parameter("mw2c", [128, R], bf16, isOutput=False)
    owt_in = nc.declare_dram_parameter("owt", [128, 4 * C], bf16, isOutput=False)
    mb0_in = nc.declare_dram_parameter("mb0", [128, 2], f32, isOutput=False)
    mb1_in = nc.declare_dram_parameter("mb1", [128, 3], f32, isOutput=False)
    mb2_in = nc.declare_dram_parameter("mb2", [128, 4], f32, isOutput=False)
    ob_in = nc.declare_dram_parameter("ob", [128, C], f32, isOutput=False)
    out_d = nc.declare_dram_parameter("out", [NP, C], f32, isOutput=True)

    # bf16 node tables: slab = this core's shard, xfull = all-gathered table
    xfull = [nc.dram_tensor(f"xfull{p}", [N // 2, 2 * H], bf16, addr_space="Shared")
             for p in range(2)]
    slab = [nc.dram_tensor(f"slab{p}", [NP, H], bf16) for p in range(2)]

    NJ = 98  # node-major 128-subchunks (last = 84 rows)

    def xfin_loc(w):
        return 64 * (w % 2), (w // 2) * WIN

    def cell_id(w, r, p):
        return (w * NRANGE + r) * 2 + p

    with tile.TileContext(nc) as tc:
        with (
            tc.tile_pool(name="cst", bufs=1) as cst,
        ):
            # ---- resident tiles ----
            eidx = cst.tile([128, ntot // 16], mybir.dt.int16)
            eind = cst.tile([128, nch * INDW], fp8)
            eoff = cst.tile([1, nch], mybir.dt.int32)
            dinv_nm = cst.tile([128, 2 * 98], f32)
            iden2 = cst.tile([128, H], bf16)
            idens = cst.tile([128, H], bf16)
            iden = cst.tile([128, 128], f32)
            wp = cst.tile([H, L * H], bf16)
            # wdd: 0.9*dinv duplicated on both partition halves
            wdd = cst.tile([128, NP], bf16)
            # x0sc: x0*(alpha/0.9)*sqrt(deg) (psum seed for the x0 residual)
            x0sc = cst.tile([H, NP], bf16)
            x_fin = cst.tile([128, ((NWIN + 1) // 2) * WIN], bf16)
            lwt = cst.tile([125, 4 * H], f32)
            lb = cst.tile([H, 1], f32)
            ones09 = cst.tile([128, H], f32)

            nc.sync.dma_start(eidx[:], eidx_in[:])
            nc.sync.dma_start(eind[:], eind_in[:])
            nc.sync.dma_start(eoff[:], eoff_in[:])
            nc.sync.dma_start(dinv_nm[:], dinv_in[:])
            nc.sync.dma_start(iden2[:], iden2_in[:])
            nc.sync.dma_start(idens[:], idens_in[:])
            nc.sync.dma_start(wp[:], wp_in[:])
            nc.sync.dma_start(lwt[:], lwt_in[:])
            nc.sync.dma_start(lb[:], lb_in[:])
            make_identity(nc, iden[:])
            nc.gpsimd.memset(ones09[:], 0.9)

            def tail_window(w, xdw, stpool, out_par, psC):
                """xdw (= x*0.9*dinv) window -> transpose -> /0.9 -> slab."""
                xb, xc = xfin_loc(w)
                winw = _winw(w)
                stage = stpool.tile([128, 4 * H], bf16, tag="stage")
                ng = (winw + 127) // 128
                for g in range(ng):
                    gn = min(128, winw - g * 128)
                    pst = psC.tile([128, H], bf16, space="PSUM", tag="psC")
                    nc.tensor.transpose(
                        pst[:gn, :],
                        xdw[xb:xb + H, g * 128: g * 128 + gn],
                        iden2[xb:xb + H, :],
                    )
                    nc.scalar.activation(
                        stage[:gn, g * H:(g + 1) * H], pst[:gn, :], AF.Copy,
                        scale=1.0 / 0.9,
                    )
                base = w * WIN
                if winw == WIN:
                    nc.sync.dma_start(
                        slab[out_par][base:base + WIN, :].rearrange(
                            "(g p) f -> p g f", p=128),
                        stage[:].rearrange("p (g f) -> p g f", f=H),
                    )
                else:
                    for g in range(ng):
                        gn = min(128, winw - g * 128)
                        nc.sync.dma_start(
                            slab[out_par][base + g * 128:base + g * 128 + gn, :],
                            stage[:gn, g * H:(g + 1) * H],
                        )

            def allgather(out_par):
                nc.gpsimd.collective_compute(
                    "AllGather", ALU.bypass,
                    replica_groups=[list(range(NCORES))],
                    ins=[slab[out_par][:]],
                    outs=[xfull[out_par][:]],
                )

            # ================= init: wdx, x0, first slab =================
            with (
                tc.tile_pool(name="initp", bufs=2) as initp,
                tc.tile_pool(name="initw", bufs=3) as initw,
                tc.tile_pool(name="ipsB", bufs=2, space="PSUM") as psB,
                tc.tile_pool(name="ipsC", bufs=2, space="PSUM") as psC,
            ):
                # wd = 0.9 * dinv broadcast, duplicated on both halves of wdd
                for j in range(NJ):
                    gn = min(128, NP - j * 128)
                    diag = initw.tile([128, 128], f32, tag="diag")
                    nc.vector.tensor_scalar_mul(
                        diag[:], iden[:], dinv_nm[:, j:j + 1])
                    pw = psB.tile([H, 128], f32, space="PSUM", tag="psB")
                    nc.tensor.matmul(pw[:, :gn], lhsT=ones09[:],
                                     rhs=diag[:, :gn], start=True, stop=True)
                    nc.scalar.activation(
                        wdd[:H, j * 128:j * 128 + gn], pw[:, :gn], AF.Copy)
                    nc.scalar.activation(
                        wdd[H:, j * 128:j * 128 + gn], pw[:, :gn], AF.Copy)
                # x0 = relu(xp @ lin0_w + b)
                for c in range(NWIN):
                    winw = _winw(c)
                    ng = (winw + 127) // 128
                    xpt = initp.tile([128, 4 * F], f32, tag="xpt")
                    base = c * WIN
                    if winw == WIN:
                        nc.sync.dma_start(
                            xpt[:].rearrange("p (g f) -> p g f", f=F),
                            xp_in[base:base + WIN, :].rearrange(
                                "(g p) f -> p g f", p=128),
                        )
                    else:
                        for g in range(ng):
                            gn = min(128, winw - g * 128)
                            nc.sync.dma_start(
                                xpt[:gn, g * F:(g + 1) * F],
                                xp_in[base + g * 128:base + g * 128 + gn, :],
                            )
                    px0 = psB.tile([H, WIN], f32, space="PSUM", tag="psB")
                    for k in range(4):
                        xt = initw.tile([125, WIN], f32, tag="xt")
                        for g in range(ng):
                            gn = min(128, winw - g * 128)
                            ptr = psC.tile([128, 128], f32, space="PSUM",
                                           tag="psC")
                            nc.tensor.transpose(
                                ptr[:125, :gn],
                                xpt[:gn, g * F + 125 * k: g * F + 125 * (k + 1)],
                                iden[:gn, :gn],
                            )
                            nc.scalar.activation(
                                xt[:, g * 128:g * 128 + gn], ptr[:125, :gn],
                                AF.Copy)
                        nc.tensor.matmul(
                            px0[:, :winw], lhsT=lwt[:, k * H:(k + 1) * H],
                            rhs=xt[:, :winw], start=(k == 0), stop=(k == 3))
                    xb, xc = xfin_loc(c)
                    nc.scalar.activation(
                        x_fin[xb:xb + H, xc:xc + winw], px0[:, :winw],
                        AF.Relu, bias=lb[:, 0:1])
                    # x0scale = (alpha/0.9)*sqrt(deg) broadcast over H rows
                    # (host bakes (alpha/0.9)/dinv; ones09 contributes 0.9,
                    # divided back out via the activation scale)
                    x0w = initw.tile([128, WIN], bf16, tag="x0w")
                    for g in range(ng):
                        gn = min(128, winw - g * 128)
                        diag2 = initw.tile([128, 128], f32, tag="diag")
                        nc.vector.tensor_scalar_mul(
                            diag2[:], iden[:],
                            dinv_nm[:, 98 + 4 * c + g:98 + 4 * c + g + 1])
                        pw2 = psB.tile([H, 128], f32, space="PSUM", tag="psB")
                        nc.tensor.matmul(pw2[:, :gn], lhsT=ones09[:],
                                         rhs=diag2[:, :gn], start=True,
                                         stop=True)
                        nc.scalar.activation(
                            x0w[xb:xb + H, g * 128:g * 128 + gn], pw2[:, :gn],
                            AF.Copy, scale=1.0 / 0.9)
                    # x0sc[:, win] = x0 * (alpha/0.9)*sqrt(deg)  (psum seed)
                    nc.vector.tensor_tensor(
                        x0sc[:, base:base + winw],
                        x_fin[xb:xb + H, xc:xc + winw],
                        x0w[xb:xb + H, :winw], op=ALU.mult)
                for w in range(NWIN):
                    xb, xc = xfin_loc(w)
                    winw = _winw(w)
                    xdw = initw.tile([128, WIN], bf16, tag="xdw")
                    nc.vector.tensor_tensor(
                        xdw[xb:xb + H, :winw],
                        x_fin[xb:xb + H, xc:xc + winw],
                        wdd[xb:xb + H, w * WIN:w * WIN + winw], op=ALU.mult)
                    tail_window(w, xdw, initw, 0, psC)
                allgather(0)

            # ================= layers =================
            with (
                tc.tile_pool(name="gp", bufs=3) as gpool,
                tc.tile_pool(name="hp", bufs=2) as hpool,
                tc.tile_pool(name="stp", bufs=2) as stpool,
                tc.tile_pool(name="psA", bufs=4, space="PSUM") as psA,
                tc.tile_pool(name="psB", bufs=2, space="PSUM") as psB,
                tc.tile_pool(name="psC", bufs=2, space="PSUM") as psC,
            ):
                cbmax = int(call_chunks.max())
                GPBUFS, PSABUFS = 3, 4
                g_hist = []
                g_hist_idx = {}
                psw_hist = []
                callno = 0
                for lay in range(nlayers):
                    par = lay % 2
                    for b in range(NBLK):
                        wlist = list(range(b * WBLOCK,
                                           min((b + 1) * WBLOCK, NWIN)))
                        gt = {}
                        gt_last = {}
                        for r in range(NRANGE):
                            cc = int(call_chunks[b][r])
                            if cc == 0:
                                continue
                            c0 = int(call_c0[b][r])
                            g = gpool.tile([128, cbmax * 128], bf16, tag="gath")
                            toks = min(TOKR, N // 2 - r * TOKR)
                            gref = nc.gpsimd.dma_gather(
                                out_ap=g[:, :cc * 128].rearrange(
                                    "p (c d) -> p c d", d=128),
                                in_ap=xfull[par][r * TOKR:r * TOKR + toks, :],
                                idxs_ap=eidx[:, c0 * 8:(c0 + cc) * 8],
                                num_idxs=cc * 128,
                                num_idxs_reg=cc * 128,
                                elem_size=2 * H,
                                single_packet=False,
                                queue_num=0 if os.environ.get("GCN_1Q")
                                else callno % 4,
                            )
                            callno += 1
                            if len(g_hist) >= GPBUFS and g_hist[-GPBUFS] is not None:
                                tile.add_dep_helper(gref.ins, g_hist[-GPBUFS].ins,
                                                    sync=False)
                            g_hist.append(None)  # placeholder, set after matmuls
                            g_hist_idx[id(g)] = len(g_hist) - 1
                            gt[r] = (g, c0)
                        pswt = {}
                        wleft = {w: sum(int(T[cell_id(w, r, p)])
                                        for r in range(NRANGE) for p in range(2))
                                 for w in wlist}
                        # seed psum with the alpha*x0 residual and the
                        # self-loop term x*dinv = (1/0.9) * (x * 0.9*dinv)
                        for w in wlist:
                            winw = _winw(w)
                            xb, xc = xfin_loc(w)
                            psw = psA.tile([H, WIN], f32, space="PSUM",
                                           tag="psw", name=f"psw{w}")
                            pswt[w] = psw
                            mm = nc.tensor.matmul(
                                psw[:, :winw], lhsT=iden2[:H, :],
                                rhs=x0sc[:, w * WIN:w * WIN + winw],
                                start=True, stop=False)
                            if len(psw_hist) >= PSABUFS:
                                tile.add_dep_helper(
                                    mm.ins, psw_hist[-PSABUFS].ins, sync=False)
                            xdw = hpool.tile([128, WIN], bf16, tag="xdw")
                            nc.vector.tensor_tensor(
                                xdw[xb:xb + H, :winw],
                                x_fin[xb:xb + H, xc:xc + winw],
                                wdd[xb:xb + H, w * WIN:w * WIN + winw],
                                op=ALU.mult)
                            nc.tensor.matmul(
                                psw[:, :winw], lhsT=idens[xb:xb + H, :],
                                rhs=xdw[xb:xb + H, :winw],
                                start=False, stop=(wleft[w] == 0))
                        for r in range(NRANGE):
                            if r not in gt:
                                continue
                            g, gchunk0 = gt[r]
                            for w in wlist:
                                psw = pswt[w]
                                for p in range(2):
                                    t = int(T[cell_id(w, r, p)])
                                    if t == 0:
                                        continue
                                    c0 = int(chunk_base[cell_id(w, r, p)])
                                    jj0 = c0 - gchunk0
                                    _, offv = nc.values_load_multi_w_load_instructions(
                                        eoff[0:1, c0:c0 + t], engines=[PE],
                                        min_val=0, max_val=WIN - INDW,
                                        skip_runtime_bounds_check=True,
                                    )
                                    for j in range(t):
                                        wleft[w] -= 1
                                        mm = nc.tensor.matmul(
                                            psw[:, ds(offv[j], INDW)],
                                            lhsT=g[:, (jj0 + j) * 128 + H * p:
                                                   (jj0 + j) * 128 + H * (p + 1)],
                                            rhs=eind[:, (c0 + j) * INDW:
                                                     (c0 + j + 1) * INDW],
                                            start=False,
                                            stop=(wleft[w] == 0),
                                        )
                                        gt_last[r] = mm
                        for w in wlist:
                            winw = _winw(w)
                            psw = pswt[w]
                            # hh = psw * 0.9*dinv ; x = relu(hh @ W'l)
                            hh = hpool.tile([H, WIN], bf16, tag="hh")
                            hm = nc.vector.tensor_tensor(
                                hh[:, :winw], psw[:, :winw],
                                wdd[:H, w * WIN:w * WIN + winw], op=ALU.mult)
                            psw_hist.append(hm)
                            ps2 = psB.tile([H, WIN], f32, space="PSUM",
                                           tag="psB")
                            nc.tensor.matmul(
                                ps2[:, :winw],
                                lhsT=wp[:, lay * H:(lay + 1) * H],
                                rhs=hh[:, :winw], start=True, stop=True)
                            xb, xc = xfin_loc(w)
                            nc.scalar.activation(
                                x_fin[xb:xb + H, xc:xc + winw],
                                ps2[:, :winw], AF.Relu)
                        for r in range(NRANGE):
                            if r in gt:
                                g, _ = gt[r]
                                g_hist[g_hist_idx[id(g)]] = gt_last.get(r)
                        if lay < nlayers - 1:
                            for w in wlist:
                                xb, xc = xfin_loc(w)
                                winw = _winw(w)
                                xdw = stpool.tile([128, WIN], bf16, tag="xdw")
                                nc.vector.tensor_tensor(
                                    xdw[xb:xb + H, :winw],
                                    x_fin[xb:xb + H, xc:xc + winw],
                                    wdd[xb:xb + H, w * WIN:w * WIN + winw],
                                    op=ALU.mult)
                                tail_window(w, xdw, stpool, 1 - par, psC)
                    if lay < nlayers - 1:
                        allgather(1 - par)

            # ================= MLP head (bf16) =================
            with (
                tc.tile_pool(name="mh", bufs=2) as mh,
                tc.tile_pool(name="mw", bufs=1) as mw,
                tc.tile_pool(name="mpsA", bufs=4, space="PSUM") as psA,
                tc.tile_pool(name="mpsB", bufs=2, space="PSUM") as psB,
                tc.tile_pool(name="mpsC", bufs=2, space="PSUM") as psC,
            ):
                w0d = mw.tile([128, M1], bf16)
                mw1a = mw.tile([128, M2], bf16)
                mw1b = mw.tile([128, M2], bf16)
                mw2a = mw.tile([128, R], bf16)
                mw2b = mw.tile([128, R], bf16)
                mw2c = mw.tile([128, R], bf16)
                owt = mw.tile([128, 4 * C], bf16)
                mb0 = mw.tile([128, 2], f32)
                mb1 = mw.tile([128, 3], f32)
                mb2 = mw.tile([128, 4], f32)
                ob = mw.tile([128, C], f32)
                nc.sync.dma_start(w0d[:], w0d_in[:])
                nc.sync.dma_start(mw1a[:], mw1a_in[:])
                nc.sync.dma_start(mw1b[:], mw1b_in[:])
                nc.sync.dma_start(mw2a[:], mw2a_in[:])
                nc.sync.dma_start(mw2b[:], mw2b_in[:])
                nc.sync.dma_start(mw2c[:], mw2c_in[:])
                nc.sync.dma_start(owt[:], owt_in[:])
                nc.sync.dma_start(mb0[:], mb0_in[:])
                nc.sync.dma_start(mb1[:], mb1_in[:])
                nc.sync.dma_start(mb2[:], mb2_in[:])
                nc.sync.dma_start(ob[:], ob_in[:])
                for c in range(NWIN):
                    winw = _winw(c)
                    xb, xc = xfin_loc(c)
                    xin = x_fin[xb:xb + H, xc:xc + winw]
                    # h1 = relu(x @ w0 + b0): [213, winw]
                    h1a = mh.tile([128, WIN], bf16, tag="h1a")
                    h1b = mh.tile([M1 - 128, WIN], bf16, tag="h1b")
                    p1 = psB.tile([128, WIN], f32, space="PSUM", tag="psB")
                    nc.tensor.matmul(p1[:, :winw], lhsT=w0d[xb:xb + H, :128],
                                     rhs=xin, start=True, stop=True)
                    nc.scalar.activation(h1a[:, :winw], p1[:, :winw], AF.Relu,
                                         bias=mb0[:, 0:1])
                    p1b = psB.tile([M1 - 128, WIN], f32, space="PSUM",
                                   tag="psB")
                    nc.tensor.matmul(p1b[:, :winw],
                                     lhsT=w0d[xb:xb + H, 128:M1],
                                     rhs=xin, start=True, stop=True)
                    nc.scalar.activation(h1b[:, :winw], p1b[:, :winw], AF.Relu,
                                         bias=mb0[:M1 - 128, 1:2])
                    # h2 = relu(h1 @ w1 + b1): [362, winw]
                    h2 = []
                    for mchunk in range(3):
                        mn = min(128, M2 - mchunk * 128)
                        p2 = psA.tile([128, WIN], f32, space="PSUM", tag="psw")
                        nc.tensor.matmul(
                            p2[:mn, :winw],
                            lhsT=mw1a[:, mchunk * 128:mchunk * 128 + mn],
                            rhs=h1a[:, :winw], start=True, stop=False)
                        nc.tensor.matmul(
                            p2[:mn, :winw],
                            lhsT=mw1b[:M1 - 128, mchunk * 128:mchunk * 128 + mn],
                            rhs=h1b[:M1 - 128, :winw], start=False, stop=True)
                        h2t = mh.tile([128, WIN], bf16, tag=f"h2_{mchunk}")
                        nc.scalar.activation(h2t[:mn, :winw], p2[:mn, :winw],
                                             AF.Relu,
                                             bias=mb1[:mn, mchunk:mchunk + 1])
                        h2.append(h2t)
                    # h3 = h2 @ w2 + b2 (no relu): [512, winw]
                    h3 = []
                    w2t = [mw2a, mw2b, mw2c]
                    kn = [128, 128, M2 - 256]
                    for mchunk in range(4):
                        p3 = psA.tile([128, WIN], f32, space="PSUM", tag="psw")
                        for k in range(3):
                            nc.tensor.matmul(
                                p3[:, :winw],
                                lhsT=w2t[k][:kn[k],
                                            mchunk * 128:(mchunk + 1) * 128],
                                rhs=h2[k][:kn[k], :winw],
                                start=(k == 0), stop=(k == 2))
                        h3t = mh.tile([128, WIN], bf16, tag=f"h3_{mchunk}")
                        nc.scalar.activation(h3t[:, :winw], p3[:, :winw],
                                             AF.Identity,
                                             bias=mb2[:, mchunk:mchunk + 1])
                        h3.append(h3t)
                    # out = h3 @ out_w + out_b: node-major [winw, 40]
                    ost = mh.tile([128, 4 * C], f32, tag="ost")
                    ng = (winw + 127) // 128
                    for g in range(ng):
                        gn = min(128, winw - g * 128)
                        po = psC.tile([128, C], f32, space="PSUM", tag="psC")
                        for k in range(4):
                            nc.tensor.matmul(
                                po[:gn, :],
                                lhsT=h3[k][:, g * 128:g * 128 + gn],
                                rhs=owt[:, k * C:(k + 1) * C],
                                start=(k == 0), stop=(k == 3))
                        nc.vector.tensor_tensor(
                            ost[:gn, g * C:(g + 1) * C], po[:gn, :],
                            ob[:gn, :], op=ALU.add)
                    base = c * WIN
                    if winw == WIN:
                        nc.sync.dma_start(
                            out_d[base:base + WIN, :].rearrange(
                                "(g p) f -> p g f", p=128),
                            ost[:].rearrange("p (g f) -> p g f", f=C),
                        )
                    else:
                        for g in range(ng):
                            gn = min(128, winw - g * 128)
                            nc.sync.dma_start(
                                out_d[base + g * 128:base + g * 128 + gn, :],
                                ost[:gn, g * C:(g + 1) * C],
                            )
    nc.finalize()
    return nc


_CACHE = {}
_LAST_EXEC_NS = None
_LAST_TRACE = None


def _get_program(meta, nlayers):
    key = (meta["nch"], nlayers, meta["T"].tobytes())
    if key not in _CACHE:
        _CACHE[key] = build_program(meta, nlayers)
    return _CACHE[key]


def kernel(**inputs):
    from concourse.bass_utils import run_bass_kernel_spmd
    edge_index = np.asarray(inputs["edge_index"])
    meta, per_core = preprocess(edge_index)
    in_maps = build_inputs(meta, per_core, inputs)
    nc = _get_program(meta, NLAYERS)
    kwargs = {}
    if os.environ.get("GCN_TRACE"):
        kwargs["trace"] = True
        if os.environ.get("GCN_TRACE_DIR"):
            os.makedirs(os.environ["GCN_TRACE_DIR"], exist_ok=True)
            kwargs["tmpdir"] = os.environ["GCN_TRACE_DIR"]
    res = run_bass_kernel_spmd(nc, in_maps, list(range(NCORES)), **kwargs)
    global _LAST_EXEC_NS, _LAST_TRACE
    if res.exec_time_ns:
        _LAST_EXEC_NS = res.exec_time_ns
        _LAST_TRACE = res.instructions_and_trace
    out = np.concatenate([res.results[i]["out"] for i in range(NCORES)], axis=0)
    return out
